# revision 7
# baseline (speedup 1.0000x reference)
"""Sparse single-head attention (QKV proj + key-padding mask + softmax) on 8 trn2 cores.

Math per batch element b (one NeuronCore each):
    qh = q @ Wq + bq ; kh = k @ Wk + bk ; vh = v @ Wv + bv        [S, 64]
    scores = qh @ kh^T / 8 ; scores[:, mask==0] = -1e10
    out = softmax(scores, -1) @ vh                                 [S, 64]

Device strategy (v2):
  - Host gathers the unmasked k/v rows (~50% of keys), casts q/k/v to bf16
    and stores them d-major (pre-transposed), so the kernel needs NO input
    transposes on the PE.  1/sqrt(64) is folded into Wq.
  - All matmuls run in bf16 (1 cycle/row at any N); accumulation stays fp32
    in PSUM, so precision loss is only the 0.4% bf16 input rounding.
  - Padded keys are neutralized without any mask bias: v is projected with an
    augmented ones-row (vg_aug = [v, 1], Wv_aug row 512 = [bv, 1]) so padded
    rows produce vh == 0 (including the ones-column used for the softmax
    denominator).  exp(score_pad) is finite garbage multiplied by zero.
  - scores are computed TRANSPOSED ([k, q] layout) with K=64 contraction,
    packed TWO key-chunks per PE pass via row tiling: chunk pairs live on
    partition halves (khT2 [128, pair, 128]) and qhT is duplicated onto both
    halves by a col-tiled projection; the two 64-row matmuls run concurrently
    in disjoint row groups of the array -> ~2x score throughput.
  - exp() is not max-stabilized: scores ~ N(0, 0.11), far inside range; the
    sum over keys comes free from the ones-column of vh (row 64 of the AV
    accumulator).
  - attn^T @ vh runs with vh natural ([key, 65]) as the stationary operand,
    K=128 full-array contraction, N=512 moving.
"""

import numpy as np
import ml_dtypes

import concourse.bass as bass
import concourse.tile as tile
from concourse import bacc, mybir
from concourse.bass_utils import run_bass_kernel_spmd
from concourse.masks import make_identity

F32 = mybir.dt.float32
BF16 = mybir.dt.bfloat16
NP_BF16 = ml_dtypes.bfloat16
S = 4096  # query rows per core
D = 512  # model dim
DK = 64  # head dim (q/k and v)
N_CORES = 8
EXP = mybir.ActivationFunctionType.Exp


def _build_nc(SK: int):
    """Build the single-core Bass program (same program on all 8 cores)."""
    assert SK % 128 == 0
    SKC = SK // 128  # 128-key chunks
    NPAIR = (SKC + 1) // 2  # chunk pairs (last may be a lone even half)
    QB = S // 512  # 512-query blocks

    nc = bacc.Bacc("TRN2", target_bir_lowering=False, debug=False)

    qt_d = nc.dram_tensor("qt", [D, S], BF16, kind="ExternalInput").ap()
    kt_d = nc.dram_tensor("kt", [D, SK], BF16, kind="ExternalInput").ap()
    vt_d = nc.dram_tensor("vt", [D, SK], BF16, kind="ExternalInput").ap()
    vones_d = nc.dram_tensor("vones", [1, SK], BF16, kind="ExternalInput").ap()
    wq_d = nc.dram_tensor("wq", [D, DK], BF16, kind="ExternalInput").ap()
    wk_d = nc.dram_tensor("wk", [D, DK], BF16, kind="ExternalInput").ap()
    wv_d = nc.dram_tensor("wv", [D, DK + 1], BF16, kind="ExternalInput").ap()
    wvl_d = nc.dram_tensor("wvl", [1, DK + 1], BF16, kind="ExternalInput").ap()
    bq_d = nc.dram_tensor("bq2", [128, 1], F32, kind="ExternalInput").ap()
    bk_d = nc.dram_tensor("bk2", [128, 1], F32, kind="ExternalInput").ap()
    out_d = nc.dram_tensor("out", [S, DK], F32, kind="ExternalOutput").ap()

    kv_blocks = []
    r0 = 0
    while r0 < SK:
        nr = min(512, SK - r0)
        kv_blocks.append((r0, nr))
        r0 += nr

    with tile.TileContext(nc) as tc:
        with (
            tc.tile_pool(name="consts", bufs=1) as consts,
            tc.tile_pool(name="persist", bufs=1) as persist,
            tc.tile_pool(name="kvp", bufs=len(kv_blocks)) as kv_pool,
            tc.tile_pool(name="qtp", bufs=3) as qt_pool,
            tc.tile_pool(name="qhp", bufs=2) as qh_pool,
            tc.tile_pool(name="etp", bufs=3) as et_pool,
            tc.tile_pool(name="stg", bufs=2) as stg_pool,
            tc.tile_pool(name="outp", bufs=2) as out_pool,
            tc.tile_pool(name="recp", bufs=4) as rec_pool,
            tc.tile_pool(name="ppa", bufs=3, space="PSUM") as ppa,
            tc.tile_pool(name="pps", bufs=2, space="PSUM") as pps,
            tc.tile_pool(name="ppo", bufs=1, space="PSUM") as ppo,
        ):
            # ---- input DMAs lead the HWDGE queue, interleaved k/v just-in-time
            NB = len(kv_blocks)
            kt_tiles = [None] * NB
            vt_tiles = [None] * NB

            def load_kv(which, bi):
                r0, nr = kv_blocks[bi]
                src_d = kt_d if which == "k" else vt_d
                t = kv_pool.tile([128, 4, 512], BF16, tag=which + "t")
                nc.sync.dma_start(
                    t[:, :, 0:nr],
                    src_d.rearrange("(c p) s -> p c s", p=128)[:, :, r0 : r0 + nr],
                )
                (kt_tiles if which == "k" else vt_tiles)[bi] = t

            kv_work = []
            ki, vi = 0, 0
            kv_work.append(("k", 0))
            ki = 1
            while ki < NB or vi < NB:
                if ki < NB:
                    kv_work.append(("k", ki))
                    ki += 1
                if vi < NB:
                    kv_work.append(("v", vi))
                    vi += 1
            for which, bi in kv_work:
                load_kv(which, bi)

            qt_tiles = [None] * QB

            def load_q(qb):
                t = qt_pool.tile([128, 4, 512], BF16, tag="qt")
                nc.sync.dma_start(
                    t[:, :, :],
                    qt_d.rearrange("(c p) s -> p c s", p=128)[
                        :, :, qb * 512 : (qb + 1) * 512
                    ],
                )
                qt_tiles[qb] = t

            load_q(0)
            load_q(1)

            # ---- consts via SWDGE (gpsimd), in consumption order (k path first)
            wq = consts.tile([128, 4, DK], BF16)
            wk = consts.tile([128, 4, DK], BF16)
            wv = consts.tile([128, 4, DK + 1], BF16)
            wvl = consts.tile([1, DK + 1], BF16)
            bq = consts.tile([128, 1], F32)
            bk = consts.tile([128, 1], F32)
            vones = consts.tile([1, SK], BF16)
            nc.gpsimd.dma_start(wk[:, :, :], wk_d.rearrange("(c p) k -> p c k", p=128))
            nc.gpsimd.dma_start(bk[:, :], bk_d)
            nc.gpsimd.dma_start(wv[:, :, :], wv_d.rearrange("(c p) k -> p c k", p=128))
            nc.gpsimd.dma_start(wvl[:, :], wvl_d)
            nc.gpsimd.dma_start(vones[:, :], vones_d)
            nc.gpsimd.dma_start(wq[:, :, :], wq_d.rearrange("(c p) k -> p c k", p=128))
            nc.gpsimd.dma_start(bq[:, :], bq_d)
            ident = consts.tile([128, 128], F32)
            make_identity(nc, ident[:, :])

            # ---- persistent K/V state ----
            khT2 = persist.tile([128, NPAIR, 128], BF16)  # pair layout, halves
            vh = persist.tile([128, SKC, DK + 1], BF16)  # natural [key, 65]

            # ---- PE warm-up: ~3.5us of dummy matmuls so HAM unthrottles ----
            pwarm = ppa.tile([64, 64], F32, tag="pa")
            for _ in range(44):
                nc.tensor.matmul(
                    pwarm[:, :], wk[:, 0, :], wk[:, 0, :], start=True, stop=True
                )

            # ---- Phase A: interleaved K / V blocks, flips delayed one item ----
            pending_flip = None

            def do_flips():
                nonlocal pending_flip
                vs, kc0, npc = pending_flip
                pending_flip = None
                pf = ppa.tile([128, 4, 128], F32, tag="pa")
                for t in range(npc):
                    nc.tensor.transpose(
                        pf[:, t, 0 : DK + 1],
                        vs[:, t * 128 : (t + 1) * 128],
                        ident[0 : DK + 1, 0 : DK + 1],
                    )
                nc.vector.tensor_copy(
                    vh[:, kc0 : kc0 + npc, :], pf[:, 0:npc, 0 : DK + 1]
                )

            for which, bi in kv_work:
                r0, nr = kv_blocks[bi]
                npc = nr // 128
                if which == "k":
                    kt = kt_tiles[bi]
                    pr0 = r0 // 256  # first pair of this block
                    # ps_k dims: [partition, local pair j, half h, cols]
                    ps_k = ppa.tile([128, 2, 2, 128], F32, tag="pa")
                    for lc in range(npc):
                        j, h = lc // 2, lc % 2
                        dst = ps_k[h * 64 : (h + 1) * 64, j, h, :]
                        for c in range(4):
                            nc.tensor.matmul(
                                dst,
                                wk[:, c, :],
                                kt[:, c, lc * 128 : (lc + 1) * 128],
                                start=(c == 0),
                                stop=(c == 3),
                                tile_position=(0, h * 64),
                            )
                    if pending_flip is not None:
                        do_flips()
                    nja = (npc + 1) // 2
                    njb = npc // 2
                    nc.vector.tensor_scalar_add(
                        khT2[0:64, pr0 : pr0 + nja, :],
                        ps_k[0:64, 0:nja, 0, :],
                        bk[0:64, :],
                    )
                    if njb:
                        nc.vector.tensor_scalar_add(
                            khT2[64:128, pr0 : pr0 + njb, :],
                            ps_k[64:128, 0:njb, 1, :],
                            bk[64:128, :],
                        )
                else:
                    vt = vt_tiles[bi]
                    kc0 = r0 // 128
                    ps_v = ppa.tile([128, 512], F32, tag="pa")
                    for c in range(4):
                        nc.tensor.matmul(
                            ps_v[0 : DK + 1, 0:nr],
                            wv[:, c, :],
                            vt[:, c, 0:nr],
                            start=(c == 0),
                            stop=False,
                        )
                    nc.tensor.matmul(
                        ps_v[0 : DK + 1, 0:nr],
                        wvl[0:1, :],
                        vones[0:1, r0 : r0 + nr],
                        start=False,
                        stop=True,
                    )
                    if pending_flip is not None:
                        do_flips()
                    vs = stg_pool.tile([DK + 1, 512], F32, tag="vs")
                    nc.vector.tensor_copy(vs[:, 0:nr], ps_v[0 : DK + 1, 0:nr])
                    pending_flip = (vs, kc0, npc)
            if pending_flip is not None:
                do_flips()

            # ---- Phase B: Q projection (duplicated halves) + attention ----
            def proj_q(qb):
                qt = qt_tiles[qb]
                ps_q = ppa.tile([128, 512], F32, tag="pa")
                for c in range(4):
                    nc.tensor.matmul(
                        ps_q[0:64, :],
                        wq[:, c, :],
                        qt[:, c, :],
                        start=(c == 0),
                        stop=(c == 3),
                        tile_position=(0, 0),
                    )
                    nc.tensor.matmul(
                        ps_q[64:128, :],
                        wq[:, c, :],
                        qt[:, c, :],
                        start=(c == 0),
                        stop=(c == 3),
                        tile_position=(0, 64),
                    )
                qh2 = qh_pool.tile([128, 512], BF16, tag="qh")
                nc.vector.tensor_scalar_add(qh2[:, :], ps_q[:, :], bq[:, :])
                return qh2

            qh_cur = proj_q(0)
            for qb in range(QB):
                if qb + 2 < QB:
                    load_q(qb + 2)
                qh_next = proj_q(qb + 1) if qb + 1 < QB else None

                po = ppo.tile([DK + 1, 512], F32, tag="po")
                prev = None
                for j in range(NPAIR):
                    both = 2 * j + 1 < SKC
                    width = 1024 if both else 512
                    ps_s = pps.tile([128, 1024], F32, tag="ss")
                    nc.tensor.matmul(
                        ps_s[:, 0:512],
                        khT2[0:64, j, :],
                        qh_cur[0:64, :],
                        start=True,
                        stop=True,
                        tile_position=(0, 0),
                    )
                    if both:
                        nc.tensor.matmul(
                            ps_s[:, 512:1024],
                            khT2[64:128, j, :],
                            qh_cur[64:128, :],
                            start=True,
                            stop=True,
                            tile_position=(64, 0),
                        )
                    et = et_pool.tile([128, 1024], BF16, tag="et")
                    nc.scalar.activation(et[:, 0:width], ps_s[:, 0:width], EXP)
                    if prev is not None:
                        pet, pj, pw = prev
                        for h in range(pw // 512):
                            kc = 2 * pj + h
                            nc.tensor.matmul(
                                po[:, :],
                                vh[:, kc, :],
                                pet[:, h * 512 : (h + 1) * 512],
                                start=(kc == 0),
                                stop=(kc == SKC - 1),
                            )
                    prev = (et, j, width)
                pet, pj, pw = prev
                for h in range(pw // 512):
                    kc = 2 * pj + h
                    nc.tensor.matmul(
                        po[:, :],
                        vh[:, kc, :],
                        pet[:, h * 512 : (h + 1) * 512],
                        start=(kc == 0),
                        stop=(kc == SKC - 1),
                    )

                # finalize: flip to natural [q, v], scale by 1/rowsum
                ot = stg_pool.tile([DK + 1, 512], F32, tag="ot")
                nc.vector.tensor_copy(ot[:, :], po[:, :])
                pf = ppa.tile([128, 4, 128], F32, tag="pa")
                for t in range(4):
                    nc.tensor.transpose(
                        pf[:, t, 0 : DK + 1],
                        ot[:, t * 128 : (t + 1) * 128],
                        ident[0 : DK + 1, 0 : DK + 1],
                    )
                rec = rec_pool.tile([128, 4, 1], F32, tag="r")
                nc.vector.reciprocal(rec[:, :, :], pf[:, :, DK : DK + 1])
                ostage = out_pool.tile([128, 4, DK], F32, tag="os")
                for t in range(4):
                    nc.vector.tensor_scalar_mul(
                        ostage[:, t, :], pf[:, t, 0:DK], rec[:, t, :]
                    )
                nc.sync.dma_start(
                    out_d[qb * 512 : (qb + 1) * 512, :].rearrange(
                        "(t p) v -> p t v", p=128
                    ),
                    ostage[:, :, :],
                )
                qh_cur = qh_next

    nc.compile()
    return nc


_NC_CACHE: dict = {}


def prepare(inputs):
    """Host-side preprocessing: returns (nc, in_maps)."""
    q = np.asarray(inputs["q"], dtype=np.float32)
    k = np.asarray(inputs["k"], dtype=np.float32)
    v = np.asarray(inputs["v"], dtype=np.float32)
    mask = np.asarray(inputs["mask"])
    Wq = np.asarray(inputs["Wq"], dtype=np.float32)
    bq = np.asarray(inputs["bq"], dtype=np.float32)
    Wk = np.asarray(inputs["Wk"], dtype=np.float32)
    bk = np.asarray(inputs["bk"], dtype=np.float32)
    Wv = np.asarray(inputs["Wv"], dtype=np.float32)
    bv = np.asarray(inputs["bv"], dtype=np.float32)
    B = q.shape[0]
    assert q.shape == (B, S, D) and B == N_CORES

    idxs = [np.nonzero(mask[b])[0] for b in range(B)]
    max_cnt = max(len(ix) for ix in idxs)
    SK = ((max_cnt + 127) // 128) * 128
    SK = max(SK, 512)

    scale = np.float32(1.0 / np.sqrt(np.float32(DK)))
    wq8 = (Wq * scale).astype(NP_BF16)
    wk8 = Wk.astype(NP_BF16)
    wv_aug = np.concatenate([Wv, np.zeros((D, 1), np.float32)], axis=1).astype(NP_BF16)
    wvl = np.concatenate([bv, np.ones(1, np.float32)]).reshape(1, DK + 1).astype(
        NP_BF16
    )
    bq2 = np.concatenate([bq * scale, bq * scale]).reshape(128, 1).astype(np.float32)
    bk2 = np.concatenate([bk, bk]).reshape(128, 1).astype(np.float32)

    in_maps = []
    for b in range(B):
        ix = idxs[b]
        cnt = len(ix)
        kt = np.zeros((D, SK), NP_BF16)
        vt = np.zeros((D, SK), NP_BF16)
        kt[:, :cnt] = k[b][ix].astype(NP_BF16).T
        vt[:, :cnt] = v[b][ix].astype(NP_BF16).T
        vones = np.zeros((1, SK), NP_BF16)
        vones[0, :cnt] = 1.0
        in_maps.append(
            dict(
                qt=np.ascontiguousarray(q[b].astype(NP_BF16).T),
                kt=np.ascontiguousarray(kt),
                vt=np.ascontiguousarray(vt),
                vones=vones,
                wq=wq8,
                wk=wk8,
                wv=wv_aug,
                wvl=wvl,
                bq2=bq2,
                bk2=bk2,
            )
        )

    if SK not in _NC_CACHE:
        _NC_CACHE[SK] = _build_nc(SK)
    return _NC_CACHE[SK], in_maps


def kernel(**inputs) -> np.ndarray:
    nc, in_maps = prepare(inputs)
    res = run_bass_kernel_spmd(nc, in_maps, list(range(N_CORES)))
    out = np.stack([res.results[b]["out"] for b in range(len(in_maps))], axis=0)
    return out.astype(np.float32)


# revision 8
# speedup vs baseline: 1.0951x; 1.0951x over previous
"""Sparse single-head attention (QKV proj + key-padding mask + softmax) on 8 trn2 cores.

Math per batch element b (one NeuronCore each):
    qh = q @ Wq + bq ; kh = k @ Wk + bk ; vh = v @ Wv + bv        [S, 64]
    scores = qh @ kh^T / 8 ; scores[:, mask==0] = -1e10
    out = softmax(scores, -1) @ vh                                 [S, 64]

Device strategy (v2):
  - Host gathers the unmasked k/v rows (~50% of keys), casts q/k/v to bf16
    and stores them d-major (pre-transposed), so the kernel needs NO input
    transposes on the PE.  1/sqrt(64) is folded into Wq.
  - All matmuls run in bf16 (1 cycle/row at any N); accumulation stays fp32
    in PSUM, so precision loss is only the 0.4% bf16 input rounding.
  - Padded keys are neutralized without any mask bias: v is projected with an
    augmented ones-row (vg_aug = [v, 1], Wv_aug row 512 = [bv, 1]) so padded
    rows produce vh == 0 (including the ones-column used for the softmax
    denominator).  exp(score_pad) is finite garbage multiplied by zero.
  - scores are computed TRANSPOSED ([k, q] layout) with K=64 contraction,
    packed TWO key-chunks per PE pass via row tiling: chunk pairs live on
    partition halves (khT2 [128, pair, 128]) and qhT is duplicated onto both
    halves by a col-tiled projection; the two 64-row matmuls run concurrently
    in disjoint row groups of the array -> ~2x score throughput.
  - exp() is not max-stabilized: scores ~ N(0, 0.11), far inside range; the
    sum over keys comes free from the ones-column of vh (row 64 of the AV
    accumulator).
  - attn^T @ vh runs with vh natural ([key, 65]) as the stationary operand,
    K=128 full-array contraction, N=512 moving.
"""

import numpy as np
import ml_dtypes

import concourse.bass as bass
import concourse.tile as tile
from concourse import bacc, mybir
from concourse.bass_utils import run_bass_kernel_spmd
from concourse.masks import make_identity

F32 = mybir.dt.float32
BF16 = mybir.dt.bfloat16
NP_BF16 = ml_dtypes.bfloat16
S = 4096  # query rows per core
D = 512  # model dim
DK = 64  # head dim (q/k and v)
N_CORES = 8
EXP = mybir.ActivationFunctionType.Exp


def _build_nc(SK: int):
    """Build the single-core Bass program (same program on all 8 cores)."""
    assert SK % 128 == 0
    SKC = SK // 128  # 128-key chunks
    NPAIR = (SKC + 1) // 2  # chunk pairs (last may be a lone even half)
    QB = S // 512  # 512-query blocks

    nc = bacc.Bacc("TRN2", target_bir_lowering=False, debug=False)

    qt_d = nc.dram_tensor("qt", [D, S], BF16, kind="ExternalInput").ap()
    kt_d = nc.dram_tensor("kt", [D, SK], BF16, kind="ExternalInput").ap()
    vt_d = nc.dram_tensor("vt", [D, SK], BF16, kind="ExternalInput").ap()
    vones_d = nc.dram_tensor("vones", [1, SK], BF16, kind="ExternalInput").ap()
    wq_d = nc.dram_tensor("wq", [D, DK], BF16, kind="ExternalInput").ap()
    wk_d = nc.dram_tensor("wk", [D, DK], BF16, kind="ExternalInput").ap()
    wv_d = nc.dram_tensor("wv", [D, DK + 1], BF16, kind="ExternalInput").ap()
    wvl_d = nc.dram_tensor("wvl", [1, DK + 1], BF16, kind="ExternalInput").ap()
    bq_d = nc.dram_tensor("bq2", [128, 1], F32, kind="ExternalInput").ap()
    bk_d = nc.dram_tensor("bk2", [128, 1], F32, kind="ExternalInput").ap()
    out_d = nc.dram_tensor("out", [S, DK], F32, kind="ExternalOutput").ap()

    kv_blocks = []
    r0 = 0
    while r0 < SK:
        nr = min(512, SK - r0)
        kv_blocks.append((r0, nr))
        r0 += nr

    with tile.TileContext(nc) as tc:
        with (
            tc.tile_pool(name="consts", bufs=1) as consts,
            tc.tile_pool(name="persist", bufs=1) as persist,
            tc.tile_pool(name="kvp", bufs=len(kv_blocks)) as kv_pool,
            tc.tile_pool(name="qtp", bufs=3) as qt_pool,
            tc.tile_pool(name="qhp", bufs=2) as qh_pool,
            tc.tile_pool(name="etp", bufs=3) as et_pool,
            tc.tile_pool(name="stg", bufs=2) as stg_pool,
            tc.tile_pool(name="outp", bufs=2) as out_pool,
            tc.tile_pool(name="recp", bufs=4) as rec_pool,
            tc.tile_pool(name="ppa", bufs=2, space="PSUM") as ppa,
            tc.tile_pool(name="pps", bufs=2, space="PSUM") as pps,
            tc.tile_pool(name="ppo", bufs=2, space="PSUM") as ppo,
        ):
            # ---- consts on the sync HWDGE queue FIRST (small, ~200KB) so
            # weights land before the bulk kt/vt streams; ident via gpsimd
            wq = consts.tile([128, 4, DK], BF16)
            wk = consts.tile([128, 4, DK], BF16)
            wv = consts.tile([128, 4, DK + 1], BF16)
            wvl = consts.tile([1, DK + 1], BF16)
            bq = consts.tile([128, 1], F32)
            bk = consts.tile([128, 1], F32)
            vones = consts.tile([1, SK], BF16)
            nc.sync.dma_start(wk[:, :, :], wk_d.rearrange("(c p) k -> p c k", p=128))
            nc.sync.dma_start(bk[:, :], bk_d)
            nc.sync.dma_start(wv[:, :, :], wv_d.rearrange("(c p) k -> p c k", p=128))
            nc.sync.dma_start(wvl[:, :], wvl_d)
            nc.sync.dma_start(vones[:, :], vones_d)
            nc.sync.dma_start(wq[:, :, :], wq_d.rearrange("(c p) k -> p c k", p=128))
            nc.sync.dma_start(bq[:, :], bq_d)
            ident = consts.tile([128, 128], F32)
            make_identity(nc, ident[:, :])

            # ---- input DMAs lead the HWDGE queue, interleaved k/v just-in-time
            NB = len(kv_blocks)
            kt_tiles = [None] * NB
            vt_tiles = [None] * NB

            def load_kv(which, bi):
                r0, nr = kv_blocks[bi]
                src_d = kt_d if which == "k" else vt_d
                t = kv_pool.tile([128, 4, 512], BF16, tag=which + "t")
                nc.sync.dma_start(
                    t[:, :, 0:nr],
                    src_d.rearrange("(c p) s -> p c s", p=128)[:, :, r0 : r0 + nr],
                )
                (kt_tiles if which == "k" else vt_tiles)[bi] = t

            kv_work = []
            ki, vi = 0, 0
            kv_work.append(("k", 0))
            ki = 1
            while ki < NB or vi < NB:
                if ki < NB:
                    kv_work.append(("k", ki))
                    ki += 1
                if vi < NB:
                    kv_work.append(("v", vi))
                    vi += 1
            for which, bi in kv_work:
                load_kv(which, bi)

            qt_tiles = [None] * QB

            def load_q(qb):
                t = qt_pool.tile([128, 4, 512], BF16, tag="qt")
                nc.sync.dma_start(
                    t[:, :, :],
                    qt_d.rearrange("(c p) s -> p c s", p=128)[
                        :, :, qb * 512 : (qb + 1) * 512
                    ],
                )
                qt_tiles[qb] = t

            load_q(0)
            load_q(1)

            # ---- persistent K/V state ----
            khT2 = persist.tile([128, NPAIR, 128], BF16)  # pair layout, halves
            vh = persist.tile([128, SKC, DK + 1], BF16)  # natural [key, 65]

            # ---- PE warm-up: ~3.5us of dummy matmuls so HAM unthrottles ----
            pwarm = ppa.tile([64, 64], F32, tag="pa")
            for _ in range(44):
                nc.tensor.matmul(
                    pwarm[:, :], wk[:, 0, :], wk[:, 0, :], start=True, stop=True
                )

            # ---- Phase A: interleaved K / V blocks, flips delayed one item ----
            pending_flip = None

            def do_flips():
                nonlocal pending_flip
                vs, kc0, npc = pending_flip
                pending_flip = None
                pf = ppa.tile([128, 4, 128], F32, tag="pa")
                for t in range(npc):
                    nc.tensor.transpose(
                        pf[:, t, 0 : DK + 1],
                        vs[:, t * 128 : (t + 1) * 128],
                        ident[0 : DK + 1, 0 : DK + 1],
                    )
                nc.vector.tensor_copy(
                    vh[:, kc0 : kc0 + npc, :], pf[:, 0:npc, 0 : DK + 1]
                )

            for which, bi in kv_work:
                r0, nr = kv_blocks[bi]
                npc = nr // 128
                if which == "k":
                    kt = kt_tiles[bi]
                    pr0 = r0 // 256  # first pair of this block
                    # ps_k dims: [partition, local pair j, half h, cols]
                    ps_k = ppa.tile([128, 2, 2, 128], F32, tag="pa")
                    for lc in range(npc):
                        j, h = lc // 2, lc % 2
                        dst = ps_k[h * 64 : (h + 1) * 64, j, h, :]
                        for c in range(4):
                            nc.tensor.matmul(
                                dst,
                                wk[:, c, :],
                                kt[:, c, lc * 128 : (lc + 1) * 128],
                                start=(c == 0),
                                stop=(c == 3),
                                tile_position=(0, h * 64),
                            )
                    if pending_flip is not None:
                        do_flips()
                    nja = (npc + 1) // 2
                    njb = npc // 2
                    nc.vector.tensor_scalar_add(
                        khT2[0:64, pr0 : pr0 + nja, :],
                        ps_k[0:64, 0:nja, 0, :],
                        bk[0:64, :],
                    )
                    if njb:
                        nc.vector.tensor_scalar_add(
                            khT2[64:128, pr0 : pr0 + njb, :],
                            ps_k[64:128, 0:njb, 1, :],
                            bk[64:128, :],
                        )
                else:
                    vt = vt_tiles[bi]
                    kc0 = r0 // 128
                    ps_v = ppa.tile([128, 512], F32, tag="pa")
                    for c in range(4):
                        nc.tensor.matmul(
                            ps_v[0 : DK + 1, 0:nr],
                            wv[:, c, :],
                            vt[:, c, 0:nr],
                            start=(c == 0),
                            stop=False,
                        )
                    nc.tensor.matmul(
                        ps_v[0 : DK + 1, 0:nr],
                        wvl[0:1, :],
                        vones[0:1, r0 : r0 + nr],
                        start=False,
                        stop=True,
                    )
                    if pending_flip is not None:
                        do_flips()
                    vs = stg_pool.tile([DK + 1, 512], F32, tag="vs")
                    nc.vector.tensor_copy(vs[:, 0:nr], ps_v[0 : DK + 1, 0:nr])
                    pending_flip = (vs, kc0, npc)
            if pending_flip is not None:
                do_flips()

            # ---- Phase B: Q projection (duplicated halves) + attention ----
            def proj_q(qb):
                qt = qt_tiles[qb]
                ps_q = ppa.tile([128, 512], F32, tag="pa")
                for c in range(4):
                    nc.tensor.matmul(
                        ps_q[0:64, :],
                        wq[:, c, :],
                        qt[:, c, :],
                        start=(c == 0),
                        stop=(c == 3),
                        tile_position=(0, 0),
                    )
                    nc.tensor.matmul(
                        ps_q[64:128, :],
                        wq[:, c, :],
                        qt[:, c, :],
                        start=(c == 0),
                        stop=(c == 3),
                        tile_position=(0, 64),
                    )
                qh2 = qh_pool.tile([128, 512], BF16, tag="qh")
                nc.vector.tensor_scalar_add(qh2[:, :], ps_q[:, :], bq[:, :])
                return qh2

            qh_cur = proj_q(0)
            for qb in range(QB):
                if qb + 2 < QB:
                    load_q(qb + 2)
                qh_next = proj_q(qb + 1) if qb + 1 < QB else None

                po = ppo.tile([DK + 1, 512], F32, tag="po")
                prev = None
                for j in range(NPAIR):
                    both = 2 * j + 1 < SKC
                    width = 1024 if both else 512
                    ps_s = pps.tile([128, 1024], F32, tag="ss")
                    nc.tensor.matmul(
                        ps_s[:, 0:512],
                        khT2[0:64, j, :],
                        qh_cur[0:64, :],
                        start=True,
                        stop=True,
                        tile_position=(0, 0),
                    )
                    if both:
                        nc.tensor.matmul(
                            ps_s[:, 512:1024],
                            khT2[64:128, j, :],
                            qh_cur[64:128, :],
                            start=True,
                            stop=True,
                            tile_position=(64, 0),
                        )
                    et = et_pool.tile([128, 1024], BF16, tag="et")
                    nc.scalar.activation(et[:, 0:width], ps_s[:, 0:width], EXP)
                    if prev is not None:
                        pet, pj, pw = prev
                        for h in range(pw // 512):
                            kc = 2 * pj + h
                            nc.tensor.matmul(
                                po[:, :],
                                vh[:, kc, :],
                                pet[:, h * 512 : (h + 1) * 512],
                                start=(kc == 0),
                                stop=(kc == SKC - 1),
                            )
                    prev = (et, j, width)
                pet, pj, pw = prev
                for h in range(pw // 512):
                    kc = 2 * pj + h
                    nc.tensor.matmul(
                        po[:, :],
                        vh[:, kc, :],
                        pet[:, h * 512 : (h + 1) * 512],
                        start=(kc == 0),
                        stop=(kc == SKC - 1),
                    )

                # finalize: flip to natural [q, v], scale by 1/rowsum
                ot = stg_pool.tile([DK + 1, 512], F32, tag="ot")
                nc.vector.tensor_copy(ot[:, :], po[:, :])
                pf = ppa.tile([128, 4, 128], F32, tag="pa")
                for t in range(4):
                    nc.tensor.transpose(
                        pf[:, t, 0 : DK + 1],
                        ot[:, t * 128 : (t + 1) * 128],
                        ident[0 : DK + 1, 0 : DK + 1],
                    )
                rec = rec_pool.tile([128, 4, 1], F32, tag="r")
                nc.vector.reciprocal(rec[:, :, :], pf[:, :, DK : DK + 1])
                ostage = out_pool.tile([128, 4, DK], F32, tag="os")
                for t in range(4):
                    nc.vector.tensor_scalar_mul(
                        ostage[:, t, :], pf[:, t, 0:DK], rec[:, t, :]
                    )
                nc.sync.dma_start(
                    out_d[qb * 512 : (qb + 1) * 512, :].rearrange(
                        "(t p) v -> p t v", p=128
                    ),
                    ostage[:, :, :],
                )
                qh_cur = qh_next

    nc.compile()
    return nc


_NC_CACHE: dict = {}


def prepare(inputs):
    """Host-side preprocessing: returns (nc, in_maps)."""
    q = np.asarray(inputs["q"], dtype=np.float32)
    k = np.asarray(inputs["k"], dtype=np.float32)
    v = np.asarray(inputs["v"], dtype=np.float32)
    mask = np.asarray(inputs["mask"])
    Wq = np.asarray(inputs["Wq"], dtype=np.float32)
    bq = np.asarray(inputs["bq"], dtype=np.float32)
    Wk = np.asarray(inputs["Wk"], dtype=np.float32)
    bk = np.asarray(inputs["bk"], dtype=np.float32)
    Wv = np.asarray(inputs["Wv"], dtype=np.float32)
    bv = np.asarray(inputs["bv"], dtype=np.float32)
    B = q.shape[0]
    assert q.shape == (B, S, D) and B == N_CORES

    idxs = [np.nonzero(mask[b])[0] for b in range(B)]
    max_cnt = max(len(ix) for ix in idxs)
    SK = ((max_cnt + 127) // 128) * 128
    SK = max(SK, 512)

    scale = np.float32(1.0 / np.sqrt(np.float32(DK)))
    wq8 = (Wq * scale).astype(NP_BF16)
    wk8 = Wk.astype(NP_BF16)
    wv_aug = np.concatenate([Wv, np.zeros((D, 1), np.float32)], axis=1).astype(NP_BF16)
    wvl = np.concatenate([bv, np.ones(1, np.float32)]).reshape(1, DK + 1).astype(
        NP_BF16
    )
    bq2 = np.concatenate([bq * scale, bq * scale]).reshape(128, 1).astype(np.float32)
    bk2 = np.concatenate([bk, bk]).reshape(128, 1).astype(np.float32)

    in_maps = []
    for b in range(B):
        ix = idxs[b]
        cnt = len(ix)
        kt = np.zeros((D, SK), NP_BF16)
        vt = np.zeros((D, SK), NP_BF16)
        kt[:, :cnt] = k[b][ix].astype(NP_BF16).T
        vt[:, :cnt] = v[b][ix].astype(NP_BF16).T
        vones = np.zeros((1, SK), NP_BF16)
        vones[0, :cnt] = 1.0
        in_maps.append(
            dict(
                qt=np.ascontiguousarray(q[b].astype(NP_BF16).T),
                kt=np.ascontiguousarray(kt),
                vt=np.ascontiguousarray(vt),
                vones=vones,
                wq=wq8,
                wk=wk8,
                wv=wv_aug,
                wvl=wvl,
                bq2=bq2,
                bk2=bk2,
            )
        )

    if SK not in _NC_CACHE:
        _NC_CACHE[SK] = _build_nc(SK)
    return _NC_CACHE[SK], in_maps


def kernel(**inputs) -> np.ndarray:
    nc, in_maps = prepare(inputs)
    res = run_bass_kernel_spmd(nc, in_maps, list(range(N_CORES)))
    out = np.stack([res.results[b]["out"] for b in range(len(in_maps))], axis=0)
    return out.astype(np.float32)


# revision 9
# speedup vs baseline: 1.1942x; 1.0904x over previous
"""Sparse single-head attention (QKV proj + key-padding mask + softmax) on 8 trn2 cores.

Math per batch element b (one NeuronCore each):
    qh = q @ Wq + bq ; kh = k @ Wk + bk ; vh = v @ Wv + bv        [S, 64]
    scores = qh @ kh^T / 8 ; scores[:, mask==0] = -1e10
    out = softmax(scores, -1) @ vh                                 [S, 64]

Device strategy (v5):
  - Host gathers the unmasked k/v rows (~50% of keys), casts q/k/v to bf16,
    pre-transposes to d-major AND tiles them [partition, block, chunk, col]
    so every device DMA is 128 contiguous per-partition descriptors (~0.7us
    issue, full-rate transfer).  1/sqrt(64) is folded into Wq.
  - All matmuls run in bf16 (1 cycle/row at any N); accumulation fp32 PSUM.
  - Padded keys neutralized with no mask bias: v is projected augmented
    (vg_aug=[v,1], Wv_aug row 512=[bv,1]) so pad rows give vh == 0 including
    the ones-column that provides the softmax denominator.
  - scores computed transposed ([k, q]), K=64 contraction packed TWO key
    chunks per PE pass via row tiling (chunk pairs on partition halves,
    qhT duplicated onto both halves by a col-tiled projection).
  - exp on ACT (the pacing engine); sum over keys from vh's ones-column
    (row 64 of the attn@V accumulator).
  - Attention software-pipelined ACROSS q-blocks: attn@V lags scores/exp by
    one chunk-pair globally, so ACT never idles at q-block boundaries.
  - DMA issue order feeds k first, then v/q just-in-time.
"""

import numpy as np
import ml_dtypes

import concourse.bass as bass
import concourse.tile as tile
from concourse import bacc, mybir
from concourse.bass_utils import run_bass_kernel_spmd
from concourse.masks import make_identity

F32 = mybir.dt.float32
BF16 = mybir.dt.bfloat16
NP_BF16 = ml_dtypes.bfloat16
S = 4096  # query rows per core
D = 512  # model dim
DK = 64  # head dim (q/k and v)
N_CORES = 8
QB = S // 512
EXP = mybir.ActivationFunctionType.Exp


def _build_nc(SK: int):
    """Build the single-core Bass program (same program on all 8 cores)."""
    assert SK % 128 == 0
    SKC = SK // 128  # 128-key chunks
    NPAIR = (SKC + 1) // 2  # chunk pairs (last may be a lone even half)
    NB = (SK + 511) // 512  # 512-key blocks

    nc = bacc.Bacc("TRN2", target_bir_lowering=False, debug=False)

    qt_d = nc.dram_tensor("qt", [128, QB * 4 * 512], BF16, kind="ExternalInput").ap()
    kt_d = nc.dram_tensor("kt", [128, NB * 4 * 512], BF16, kind="ExternalInput").ap()
    vt_d = nc.dram_tensor("vt", [128, NB * 4 * 512], BF16, kind="ExternalInput").ap()
    vones_d = nc.dram_tensor("vones", [1, SK], BF16, kind="ExternalInput").ap()
    wq_d = nc.dram_tensor("wq", [D, DK], BF16, kind="ExternalInput").ap()
    wk_d = nc.dram_tensor("wk", [D, DK], BF16, kind="ExternalInput").ap()
    wv_d = nc.dram_tensor("wv", [D, DK + 1], BF16, kind="ExternalInput").ap()
    wvl_d = nc.dram_tensor("wvl", [1, DK + 1], BF16, kind="ExternalInput").ap()
    bq_d = nc.dram_tensor("bq2", [128, 1], F32, kind="ExternalInput").ap()
    bk_d = nc.dram_tensor("bk2", [128, 1], F32, kind="ExternalInput").ap()
    out_d = nc.dram_tensor("out", [S, DK], F32, kind="ExternalOutput").ap()

    kv_blocks = []
    r0 = 0
    while r0 < SK:
        nr = min(512, SK - r0)
        kv_blocks.append((r0, nr))
        r0 += nr
    assert len(kv_blocks) == NB

    with tile.TileContext(nc) as tc:
        with (
            tc.tile_pool(name="consts", bufs=1) as consts,
            tc.tile_pool(name="persist", bufs=1) as persist,
            tc.tile_pool(name="qhp", bufs=2) as qh_pool,
            tc.tile_pool(name="etp", bufs=3) as et_pool,
            tc.tile_pool(name="stg", bufs=2) as stg_pool,
            tc.tile_pool(name="outp", bufs=2) as out_pool,
            tc.tile_pool(name="recp", bufs=4) as rec_pool,
            tc.tile_pool(name="ppa", bufs=2, space="PSUM") as ppa,
            tc.tile_pool(name="pps", bufs=2, space="PSUM") as pps,
            tc.tile_pool(name="ppo", bufs=2, space="PSUM") as ppo,
        ):
            # ---- consts on the sync HWDGE queue first (small, fast) ----
            wq = consts.tile([128, 4, DK], BF16)
            wk = consts.tile([128, 4, DK], BF16)
            wv = consts.tile([128, 4, DK + 1], BF16)
            wvl = consts.tile([1, DK + 1], BF16)
            bq = consts.tile([128, 1], F32)
            bk = consts.tile([128, 1], F32)
            vones = consts.tile([1, SK], BF16)
            nc.sync.dma_start(wk[:, :, :], wk_d.rearrange("(c p) k -> p c k", p=128))
            nc.sync.dma_start(bk[:, :], bk_d)
            nc.sync.dma_start(wq[:, :, :], wq_d.rearrange("(c p) k -> p c k", p=128))
            nc.sync.dma_start(bq[:, :], bq_d)
            nc.sync.dma_start(wv[:, :, :], wv_d.rearrange("(c p) k -> p c k", p=128))
            nc.sync.dma_start(wvl[:, :], wvl_d)
            nc.sync.dma_start(vones[:, :], vones_d)
            ident = consts.tile([128, 128], F32)
            make_identity(nc, ident[:, :])

            # ---- resident input tiles (whole tensors stay in SBUF) ----
            kt_all = persist.tile([128, NB, 4, 512], BF16)
            vt_all = persist.tile([128, NB, 4, 512], BF16)
            qt_all = persist.tile([128, QB, 4, 512], BF16)
            kt_v = kt_d.rearrange("p (b c s) -> p b c s", b=NB, c=4)
            vt_v = vt_d.rearrange("p (b c s) -> p b c s", b=NB, c=4)
            qt_v = qt_d.rearrange("p (b c s) -> p b c s", b=QB, c=4)

            def load_kv(which, bi):
                src = kt_v if which == "k" else vt_v
                dst = kt_all if which == "k" else vt_all
                nc.sync.dma_start(dst[:, bi, :, :], src[:, bi, :, :])

            def load_q(qb):
                nc.sync.dma_start(qt_all[:, qb, :, :], qt_v[:, qb, :, :])

            # DMA issue order: k leads, v and q just-in-time
            kq = list(range(NB))
            vq = list(range(NB))
            qq = list(range(QB))
            load_kv("k", kq.pop(0))
            load_kv("k", kq.pop(0))
            load_kv("v", vq.pop(0))
            load_q(qq.pop(0))
            step = 0
            while kq or vq:
                if kq:
                    load_kv("k", kq.pop(0))
                if vq:
                    load_kv("v", vq.pop(0))
                step += 1
                if step == 2 and qq:
                    load_q(qq.pop(0))
            while qq:
                load_q(qq.pop(0))

            # ---- persistent K/V state ----
            khT2 = persist.tile([128, NPAIR, 128], BF16)  # pair layout, halves
            vh = persist.tile([128, SKC, DK + 1], BF16)  # natural [key, 65]

            # ---- Phase A: interleaved K / V blocks, flips delayed one item ----
            kv_work = []
            ki, vi = 0, 0
            kv_work.append(("k", 0))
            ki = 1
            while ki < NB or vi < NB:
                if ki < NB:
                    kv_work.append(("k", ki))
                    ki += 1
                if vi < NB:
                    kv_work.append(("v", vi))
                    vi += 1

            pending_flip = None

            def do_flips():
                nonlocal pending_flip
                vs, kc0, npc = pending_flip
                pending_flip = None
                pf = ppa.tile([128, 4, 128], F32, tag="pa")
                for t in range(npc):
                    nc.tensor.transpose(
                        pf[:, t, 0 : DK + 1],
                        vs[:, t * 128 : (t + 1) * 128],
                        ident[0 : DK + 1, 0 : DK + 1],
                    )
                nc.vector.tensor_copy(
                    vh[:, kc0 : kc0 + npc, :], pf[:, 0:npc, 0 : DK + 1]
                )

            def proj_q(qb):
                ps_q = ppa.tile([128, 512], F32, tag="pa")
                for c in range(4):
                    nc.tensor.matmul(
                        ps_q[0:64, :],
                        wq[:, c, :],
                        qt_all[:, qb, c, :],
                        start=(c == 0),
                        stop=(c == 3),
                        tile_position=(0, 0),
                    )
                    nc.tensor.matmul(
                        ps_q[64:128, :],
                        wq[:, c, :],
                        qt_all[:, qb, c, :],
                        start=(c == 0),
                        stop=(c == 3),
                        tile_position=(0, 64),
                    )
                qh2 = qh_pool.tile([128, 512], BF16, tag="qh")
                nc.vector.tensor_scalar_add(qh2[:, :], ps_q[:, :], bq[:, :])
                return qh2

            qh_first = None
            for wi, (which, bi) in enumerate(kv_work):
                r0, nr = kv_blocks[bi]
                npc = nr // 128
                if which == "k":
                    pr0 = r0 // 256  # first pair of this block
                    # ps_k dims: [partition, local pair j, half h, cols]
                    ps_k = ppa.tile([128, 2, 2, 128], F32, tag="pa")
                    for lc in range(npc):
                        j, h = lc // 2, lc % 2
                        dst = ps_k[h * 64 : (h + 1) * 64, j, h, :]
                        for c in range(4):
                            nc.tensor.matmul(
                                dst,
                                wk[:, c, :],
                                kt_all[:, bi, c, lc * 128 : (lc + 1) * 128],
                                start=(c == 0),
                                stop=(c == 3),
                                tile_position=(0, h * 64),
                            )
                    if pending_flip is not None:
                        do_flips()
                    nja = (npc + 1) // 2
                    njb = npc // 2
                    nc.vector.tensor_scalar_add(
                        khT2[0:64, pr0 : pr0 + nja, :],
                        ps_k[0:64, 0:nja, 0, :],
                        bk[0:64, :],
                    )
                    if njb:
                        nc.vector.tensor_scalar_add(
                            khT2[64:128, pr0 : pr0 + njb, :],
                            ps_k[64:128, 0:njb, 1, :],
                            bk[64:128, :],
                        )
                else:
                    kc0 = r0 // 128
                    ps_v = ppa.tile([128, 512], F32, tag="pa")
                    for c in range(4):
                        nc.tensor.matmul(
                            ps_v[0 : DK + 1, 0:nr],
                            wv[:, c, :],
                            vt_all[:, bi, c, 0:nr],
                            start=(c == 0),
                            stop=False,
                        )
                    nc.tensor.matmul(
                        ps_v[0 : DK + 1, 0:nr],
                        wvl[0:1, :],
                        vones[0:1, r0 : r0 + nr],
                        start=False,
                        stop=True,
                    )
                    if pending_flip is not None:
                        do_flips()
                    vs = stg_pool.tile([DK + 1, 512], F32, tag="vs")
                    nc.vector.tensor_copy(vs[:, 0:nr], ps_v[0 : DK + 1, 0:nr])
                    pending_flip = (vs, kc0, npc)
                if wi == 2 and qh_first is None:
                    # q-proj for qb0 early, between phase-A blocks
                    qh_first = proj_q(0)
            if pending_flip is not None:
                do_flips()
            if qh_first is None:
                qh_first = proj_q(0)

            # ---- Phase B: global pipeline over (qb, pair); AV lags by one ----
            def av(state):
                et, sqb, j, width, spo = state
                for h in range(width // 512):
                    kc = 2 * j + h
                    nc.tensor.matmul(
                        spo[:, :],
                        vh[:, kc, :],
                        et[:, h * 512 : (h + 1) * 512],
                        start=(kc == 0),
                        stop=(kc == SKC - 1),
                    )

            def finalize(sqb, spo):
                ot = stg_pool.tile([DK + 1, 512], F32, tag="ot")
                nc.vector.tensor_copy(ot[:, :], spo[:, :])
                pf = ppa.tile([128, 4, 128], F32, tag="pa")
                for t in range(4):
                    nc.tensor.transpose(
                        pf[:, t, 0 : DK + 1],
                        ot[:, t * 128 : (t + 1) * 128],
                        ident[0 : DK + 1, 0 : DK + 1],
                    )
                rec = rec_pool.tile([128, 4, 1], F32, tag="r")
                nc.vector.reciprocal(rec[:, :, :], pf[:, :, DK : DK + 1])
                ostage = out_pool.tile([128, 4, DK], F32, tag="os")
                for t in range(4):
                    nc.vector.tensor_scalar_mul(
                        ostage[:, t, :], pf[:, t, 0:DK], rec[:, t, :]
                    )
                nc.sync.dma_start(
                    out_d[sqb * 512 : (sqb + 1) * 512, :].rearrange(
                        "(t p) v -> p t v", p=128
                    ),
                    ostage[:, :, :],
                )

            qh_cur = qh_first
            qh_next = None
            prev = None  # (et, qb, j, width, po)
            for qb in range(QB):
                po = ppo.tile([DK + 1, 512], F32, tag="po")
                if qb + 1 < QB:
                    qh_next = proj_q(qb + 1)
                for j in range(NPAIR):
                    both = 2 * j + 1 < SKC
                    width = 1024 if both else 512
                    ps_s = pps.tile([128, 1024], F32, tag="ss")
                    nc.tensor.matmul(
                        ps_s[:, 0:512],
                        khT2[0:64, j, :],
                        qh_cur[0:64, :],
                        start=True,
                        stop=True,
                        tile_position=(0, 0),
                    )
                    if both:
                        nc.tensor.matmul(
                            ps_s[:, 512:1024],
                            khT2[64:128, j, :],
                            qh_cur[64:128, :],
                            start=True,
                            stop=True,
                            tile_position=(64, 0),
                        )
                    et = et_pool.tile([128, 1024], BF16, tag="et")
                    nc.scalar.activation(et[:, 0:width], ps_s[:, 0:width], EXP)
                    if prev is not None:
                        av(prev)
                        if prev[2] == NPAIR - 1:  # last pair of its q-block
                            finalize(prev[1], prev[4])
                    prev = (et, qb, j, width, po)
                qh_cur = qh_next
            av(prev)
            finalize(prev[1], prev[4])

    nc.compile()
    return nc


_NC_CACHE: dict = {}


def _tile_dmajor(xT: np.ndarray, nblk: int) -> np.ndarray:
    """[512, L] d-major -> [128, nblk*4*512] with per-partition contiguous
    (block, chunk, col) layout; L is zero-padded to nblk*512."""
    L = xT.shape[1]
    tmp = np.zeros((4, 128, nblk * 512), NP_BF16)
    tmp[:, :, :L] = xT.reshape(4, 128, L)
    return np.ascontiguousarray(
        tmp.reshape(4, 128, nblk, 512).transpose(1, 2, 0, 3).reshape(128, -1)
    )


def prepare(inputs):
    """Host-side preprocessing: returns (nc, in_maps)."""
    q = np.asarray(inputs["q"], dtype=np.float32)
    k = np.asarray(inputs["k"], dtype=np.float32)
    v = np.asarray(inputs["v"], dtype=np.float32)
    mask = np.asarray(inputs["mask"])
    Wq = np.asarray(inputs["Wq"], dtype=np.float32)
    bq = np.asarray(inputs["bq"], dtype=np.float32)
    Wk = np.asarray(inputs["Wk"], dtype=np.float32)
    bk = np.asarray(inputs["bk"], dtype=np.float32)
    Wv = np.asarray(inputs["Wv"], dtype=np.float32)
    bv = np.asarray(inputs["bv"], dtype=np.float32)
    B = q.shape[0]
    assert q.shape == (B, S, D) and B == N_CORES

    idxs = [np.nonzero(mask[b])[0] for b in range(B)]
    max_cnt = max(len(ix) for ix in idxs)
    SK = ((max_cnt + 127) // 128) * 128
    SK = max(SK, 512)
    NB = (SK + 511) // 512

    scale = np.float32(1.0 / np.sqrt(np.float32(DK)))
    wq8 = (Wq * scale).astype(NP_BF16)
    wk8 = Wk.astype(NP_BF16)
    wv_aug = np.concatenate([Wv, np.zeros((D, 1), np.float32)], axis=1).astype(NP_BF16)
    wvl = np.concatenate([bv, np.ones(1, np.float32)]).reshape(1, DK + 1).astype(
        NP_BF16
    )
    bq2 = np.concatenate([bq * scale, bq * scale]).reshape(128, 1).astype(np.float32)
    bk2 = np.concatenate([bk, bk]).reshape(128, 1).astype(np.float32)

    in_maps = []
    for b in range(B):
        ix = idxs[b]
        cnt = len(ix)
        ktb = np.zeros((D, SK), NP_BF16)
        vtb = np.zeros((D, SK), NP_BF16)
        ktb[:, :cnt] = k[b][ix].astype(NP_BF16).T
        vtb[:, :cnt] = v[b][ix].astype(NP_BF16).T
        vones = np.zeros((1, SK), NP_BF16)
        vones[0, :cnt] = 1.0
        in_maps.append(
            dict(
                qt=_tile_dmajor(np.ascontiguousarray(q[b].astype(NP_BF16).T), QB),
                kt=_tile_dmajor(ktb, NB),
                vt=_tile_dmajor(vtb, NB),
                vones=vones,
                wq=wq8,
                wk=wk8,
                wv=wv_aug,
                wvl=wvl,
                bq2=bq2,
                bk2=bk2,
            )
        )

    if SK not in _NC_CACHE:
        _NC_CACHE[SK] = _build_nc(SK)
    return _NC_CACHE[SK], in_maps


def kernel(**inputs) -> np.ndarray:
    nc, in_maps = prepare(inputs)
    res = run_bass_kernel_spmd(nc, in_maps, list(range(N_CORES)))
    out = np.stack([res.results[b]["out"] for b in range(len(in_maps))], axis=0)
    return out.astype(np.float32)


# revision 10
# speedup vs baseline: 1.2068x; 1.0106x over previous
"""Sparse single-head attention (QKV proj + key-padding mask + softmax) on 8 trn2 cores.

Math per batch element b (one NeuronCore each):
    qh = q @ Wq + bq ; kh = k @ Wk + bk ; vh = v @ Wv + bv        [S, 64]
    scores = qh @ kh^T / 8 ; scores[:, mask==0] = -1e10
    out = softmax(scores, -1) @ vh                                 [S, 64]

Device strategy (v5):
  - Host gathers the unmasked k/v rows (~50% of keys), casts q/k/v to bf16,
    pre-transposes to d-major AND tiles them [partition, block, chunk, col]
    so every device DMA is 128 contiguous per-partition descriptors (~0.7us
    issue, full-rate transfer).  1/sqrt(64) is folded into Wq.
  - All matmuls run in bf16 (1 cycle/row at any N); accumulation fp32 PSUM.
  - Padded keys neutralized with no mask bias: v is projected augmented
    (vg_aug=[v,1], Wv_aug row 512=[bv,1]) so pad rows give vh == 0 including
    the ones-column that provides the softmax denominator.
  - scores computed transposed ([k, q]), K=64 contraction packed TWO key
    chunks per PE pass via row tiling (chunk pairs on partition halves,
    qhT duplicated onto both halves by a col-tiled projection).
  - exp on ACT (the pacing engine); sum over keys from vh's ones-column
    (row 64 of the attn@V accumulator).
  - Attention software-pipelined ACROSS q-blocks: attn@V lags scores/exp by
    one chunk-pair globally, so ACT never idles at q-block boundaries.
  - DMA issue order feeds k first, then v/q just-in-time.
"""

import numpy as np
import ml_dtypes

import concourse.bass as bass
import concourse.tile as tile
from concourse import bacc, mybir
from concourse.bass_utils import run_bass_kernel_spmd
from concourse.masks import make_identity

F32 = mybir.dt.float32
BF16 = mybir.dt.bfloat16
NP_BF16 = ml_dtypes.bfloat16
S = 4096  # query rows per core
D = 512  # model dim
DK = 64  # head dim (q/k and v)
N_CORES = 8
QB = S // 512
EXP = mybir.ActivationFunctionType.Exp
A_EXP = float(128.0 / np.log(2.0))  # Schraudolph bf16 scale
B_EXP = 16250.56  # bias centered to zero mean relative error


def _build_nc(SK: int):
    """Build the single-core Bass program (same program on all 8 cores)."""
    assert SK % 128 == 0
    SKC = SK // 128  # 128-key chunks
    NPAIR = (SKC + 1) // 2  # chunk pairs (last may be a lone even half)
    NB = (SK + 511) // 512  # 512-key blocks

    nc = bacc.Bacc("TRN2", target_bir_lowering=False, debug=False)

    qt_d = nc.dram_tensor("qt", [128, QB * 4 * 512], BF16, kind="ExternalInput").ap()
    kt_d = nc.dram_tensor("kt", [128, NB * 4 * 512], BF16, kind="ExternalInput").ap()
    vt_d = nc.dram_tensor("vt", [128, NB * 4 * 512], BF16, kind="ExternalInput").ap()
    blob_d = nc.dram_tensor("blob", [128, 772], BF16, kind="ExternalInput").ap()
    blob2_d = nc.dram_tensor("blob2", [1, DK + 1 + SK], BF16, kind="ExternalInput").ap()
    blobf_d = nc.dram_tensor("blobf", [128, 2], F32, kind="ExternalInput").ap()
    out_d = nc.dram_tensor("out", [S, DK], F32, kind="ExternalOutput").ap()

    kv_blocks = []
    r0 = 0
    while r0 < SK:
        nr = min(512, SK - r0)
        kv_blocks.append((r0, nr))
        r0 += nr
    assert len(kv_blocks) == NB

    with tile.TileContext(nc) as tc:
        with (
            tc.tile_pool(name="consts", bufs=1) as consts,
            tc.tile_pool(name="persist", bufs=1) as persist,
            tc.tile_pool(name="qhp", bufs=2) as qh_pool,
            tc.tile_pool(name="etp", bufs=3) as et_pool,
            tc.tile_pool(name="stg", bufs=2) as stg_pool,
            tc.tile_pool(name="outp", bufs=2) as out_pool,
            tc.tile_pool(name="recp", bufs=4) as rec_pool,
            tc.tile_pool(name="ppa", bufs=2, space="PSUM") as ppa,
            tc.tile_pool(name="pps", bufs=2, space="PSUM") as pps,
            tc.tile_pool(name="ppo", bufs=2, space="PSUM") as ppo,
        ):
            # ---- consts: three packed DMAs (blob, row-blob, f32 biases) ----
            blob = consts.tile([128, 772], BF16)
            blob2 = consts.tile([1, DK + 1 + SK], BF16)
            blobf = consts.tile([128, 2], F32)
            nc.sync.dma_start(blob[:, :], blob_d)
            nc.sync.dma_start(blob2[:, :], blob2_d)
            nc.sync.dma_start(blobf[:, :], blobf_d)

            def wkc(c):
                return blob[:, c * 64 : (c + 1) * 64]

            def wqc(c):
                return blob[:, 256 + c * 64 : 256 + (c + 1) * 64]

            def wvc(c):
                return blob[:, 512 + c * 65 : 512 + (c + 1) * 65]

            wvl = blob2[0:1, 0 : DK + 1]
            vones = blob2[0:1, DK + 1 :]
            bq = blobf[:, 0:1]
            bk = blobf[:, 1:2]
            ident = consts.tile([128, 128], F32)
            make_identity(nc, ident[:, :])

            # ---- resident input tiles (whole tensors stay in SBUF) ----
            kt_all = persist.tile([128, NB, 4, 512], BF16)
            vt_all = persist.tile([128, NB, 4, 512], BF16)
            qt_all = persist.tile([128, QB, 4, 512], BF16)
            kt_v = kt_d.rearrange("p (b c s) -> p b c s", b=NB, c=4)
            vt_v = vt_d.rearrange("p (b c s) -> p b c s", b=NB, c=4)
            qt_v = qt_d.rearrange("p (b c s) -> p b c s", b=QB, c=4)

            def load_kv(which, bi):
                src = kt_v if which == "k" else vt_v
                dst = kt_all if which == "k" else vt_all
                nc.sync.dma_start(dst[:, bi, :, :], src[:, bi, :, :])

            def load_q(qb):
                nc.sync.dma_start(qt_all[:, qb, :, :], qt_v[:, qb, :, :])

            # DMA issue order: k leads, v and q just-in-time
            kq = list(range(NB))
            vq = list(range(NB))
            qq = list(range(QB))
            load_kv("k", kq.pop(0))
            load_kv("k", kq.pop(0))
            load_kv("v", vq.pop(0))
            load_q(qq.pop(0))
            step = 0
            while kq or vq:
                if kq:
                    load_kv("k", kq.pop(0))
                if vq:
                    load_kv("v", vq.pop(0))
                step += 1
                if step == 2 and qq:
                    load_q(qq.pop(0))
            while qq:
                load_q(qq.pop(0))

            # ---- persistent K/V state ----
            khT2 = persist.tile([128, NPAIR, 128], BF16)  # pair layout, halves
            vh = persist.tile([128, SKC, DK + 1], BF16)  # natural [key, 65]

            # ---- Phase A: interleaved K / V blocks, flips delayed one item ----
            kv_work = []
            ki, vi = 0, 0
            kv_work.append(("k", 0))
            ki = 1
            while ki < NB or vi < NB:
                if ki < NB:
                    kv_work.append(("k", ki))
                    ki += 1
                if vi < NB:
                    kv_work.append(("v", vi))
                    vi += 1

            pending_flip = None

            def do_flips():
                nonlocal pending_flip
                vs, kc0, npc = pending_flip
                pending_flip = None
                pf = ppa.tile([128, 4, 128], F32, tag="pa")
                for t in range(npc):
                    nc.tensor.transpose(
                        pf[:, t, 0 : DK + 1],
                        vs[:, t * 128 : (t + 1) * 128],
                        ident[0 : DK + 1, 0 : DK + 1],
                    )
                nc.vector.tensor_copy(
                    vh[:, kc0 : kc0 + npc, :], pf[:, 0:npc, 0 : DK + 1]
                )

            def proj_q(qb):
                ps_q = ppa.tile([128, 512], F32, tag="pa")
                for c in range(4):
                    nc.tensor.matmul(
                        ps_q[0:64, :],
                        wqc(c),
                        qt_all[:, qb, c, :],
                        start=(c == 0),
                        stop=(c == 3),
                        tile_position=(0, 0),
                    )
                    nc.tensor.matmul(
                        ps_q[64:128, :],
                        wqc(c),
                        qt_all[:, qb, c, :],
                        start=(c == 0),
                        stop=(c == 3),
                        tile_position=(0, 64),
                    )
                qh2 = qh_pool.tile([128, 512], BF16, tag="qh")
                nc.vector.tensor_scalar_add(qh2[:, :], ps_q[:, :], bq[:, :])
                return qh2

            qh_first = None
            for wi, (which, bi) in enumerate(kv_work):
                r0, nr = kv_blocks[bi]
                npc = nr // 128
                if which == "k":
                    pr0 = r0 // 256  # first pair of this block
                    # ps_k dims: [partition, local pair j, half h, cols]
                    ps_k = ppa.tile([128, 2, 2, 128], F32, tag="pa")
                    for lc in range(npc):
                        j, h = lc // 2, lc % 2
                        dst = ps_k[h * 64 : (h + 1) * 64, j, h, :]
                        for c in range(4):
                            nc.tensor.matmul(
                                dst,
                                wkc(c),
                                kt_all[:, bi, c, lc * 128 : (lc + 1) * 128],
                                start=(c == 0),
                                stop=(c == 3),
                                tile_position=(0, h * 64),
                            )
                    if pending_flip is not None:
                        do_flips()
                    nja = (npc + 1) // 2
                    njb = npc // 2
                    nc.vector.tensor_scalar_add(
                        khT2[0:64, pr0 : pr0 + nja, :],
                        ps_k[0:64, 0:nja, 0, :],
                        bk[0:64, :],
                    )
                    if njb:
                        nc.vector.tensor_scalar_add(
                            khT2[64:128, pr0 : pr0 + njb, :],
                            ps_k[64:128, 0:njb, 1, :],
                            bk[64:128, :],
                        )
                else:
                    kc0 = r0 // 128
                    ps_v = ppa.tile([128, 512], F32, tag="pa")
                    for c in range(4):
                        nc.tensor.matmul(
                            ps_v[0 : DK + 1, 0:nr],
                            wvc(c),
                            vt_all[:, bi, c, 0:nr],
                            start=(c == 0),
                            stop=False,
                        )
                    nc.tensor.matmul(
                        ps_v[0 : DK + 1, 0:nr],
                        wvl,
                        vones[0:1, r0 : r0 + nr],
                        start=False,
                        stop=True,
                    )
                    if pending_flip is not None:
                        do_flips()
                    vs = stg_pool.tile([DK + 1, 512], F32, tag="vs")
                    nc.vector.tensor_copy(vs[:, 0:nr], ps_v[0 : DK + 1, 0:nr])
                    pending_flip = (vs, kc0, npc)
                if wi == 2 and qh_first is None:
                    # q-proj for qb0 early, between phase-A blocks
                    qh_first = proj_q(0)
            if pending_flip is not None:
                do_flips()
            if qh_first is None:
                qh_first = proj_q(0)

            # last pairs of each q-block exp on DVE (fast-exp), rest on ACT
            dve_pairs = set(range(max(0, NPAIR - 3), NPAIR)) if NPAIR >= 6 else set()

            # ---- Phase B: global pipeline over (qb, pair); AV lags by one ----
            def av(state):
                et, sqb, j, width, spo = state
                for h in range(width // 512):
                    kc = 2 * j + h
                    nc.tensor.matmul(
                        spo[:, :],
                        vh[:, kc, :],
                        et[:, h * 512 : (h + 1) * 512],
                        start=(kc == 0),
                        stop=(kc == SKC - 1),
                    )

            def finalize(sqb, spo):
                ot = stg_pool.tile([DK + 1, 512], F32, tag="ot")
                nc.vector.tensor_copy(ot[:, :], spo[:, :])
                pf = ppa.tile([128, 4, 128], F32, tag="pa")
                for t in range(4):
                    nc.tensor.transpose(
                        pf[:, t, 0 : DK + 1],
                        ot[:, t * 128 : (t + 1) * 128],
                        ident[0 : DK + 1, 0 : DK + 1],
                    )
                rec = rec_pool.tile([128, 4, 1], F32, tag="r")
                nc.vector.reciprocal(rec[:, :, :], pf[:, :, DK : DK + 1])
                ostage = out_pool.tile([128, 4, DK], F32, tag="os")
                for t in range(4):
                    nc.vector.tensor_scalar_mul(
                        ostage[:, t, :], pf[:, t, 0:DK], rec[:, t, :]
                    )
                nc.sync.dma_start(
                    out_d[sqb * 512 : (sqb + 1) * 512, :].rearrange(
                        "(t p) v -> p t v", p=128
                    ),
                    ostage[:, :, :],
                )

            qh_cur = qh_first
            qh_next = None
            prev = None  # (et, qb, j, width, po)
            for qb in range(QB):
                po = ppo.tile([DK + 1, 512], F32, tag="po")
                for j in range(NPAIR):
                    if j == min(2, NPAIR - 1) and qb + 1 < QB:
                        qh_next = proj_q(qb + 1)
                    both = 2 * j + 1 < SKC
                    width = 1024 if both else 512
                    ps_s = pps.tile([128, 1024], F32, tag="ss")
                    nc.tensor.matmul(
                        ps_s[:, 0:512],
                        khT2[0:64, j, :],
                        qh_cur[0:64, :],
                        start=True,
                        stop=True,
                        tile_position=(0, 0),
                    )
                    if both:
                        nc.tensor.matmul(
                            ps_s[:, 512:1024],
                            khT2[64:128, j, :],
                            qh_cur[64:128, :],
                            start=True,
                            stop=True,
                            tile_position=(64, 0),
                        )
                    et = et_pool.tile([128, 1024], BF16, tag="et")
                    if j in dve_pairs:
                        nc.vector.tensor_scalar(
                            et[:, 0:width].bitcast(mybir.dt.int16),
                            ps_s[:, 0:width],
                            A_EXP,
                            B_EXP,
                            mybir.AluOpType.mult,
                            mybir.AluOpType.add,
                        )
                    else:
                        nc.scalar.activation(et[:, 0:width], ps_s[:, 0:width], EXP)
                    if prev is not None:
                        av(prev)
                        if prev[2] == NPAIR - 1:  # last pair of its q-block
                            finalize(prev[1], prev[4])
                    prev = (et, qb, j, width, po)
                qh_cur = qh_next
            av(prev)
            finalize(prev[1], prev[4])

    nc.compile()
    return nc


_NC_CACHE: dict = {}


def _tile_dmajor(xT: np.ndarray, nblk: int) -> np.ndarray:
    """[512, L] d-major -> [128, nblk*4*512] with per-partition contiguous
    (block, chunk, col) layout; L is zero-padded to nblk*512."""
    L = xT.shape[1]
    tmp = np.zeros((4, 128, nblk * 512), NP_BF16)
    tmp[:, :, :L] = xT.reshape(4, 128, L)
    return np.ascontiguousarray(
        tmp.reshape(4, 128, nblk, 512).transpose(1, 2, 0, 3).reshape(128, -1)
    )


def prepare(inputs):
    """Host-side preprocessing: returns (nc, in_maps)."""
    q = np.asarray(inputs["q"], dtype=np.float32)
    k = np.asarray(inputs["k"], dtype=np.float32)
    v = np.asarray(inputs["v"], dtype=np.float32)
    mask = np.asarray(inputs["mask"])
    Wq = np.asarray(inputs["Wq"], dtype=np.float32)
    bq = np.asarray(inputs["bq"], dtype=np.float32)
    Wk = np.asarray(inputs["Wk"], dtype=np.float32)
    bk = np.asarray(inputs["bk"], dtype=np.float32)
    Wv = np.asarray(inputs["Wv"], dtype=np.float32)
    bv = np.asarray(inputs["bv"], dtype=np.float32)
    B = q.shape[0]
    assert q.shape == (B, S, D) and B == N_CORES

    idxs = [np.nonzero(mask[b])[0] for b in range(B)]
    max_cnt = max(len(ix) for ix in idxs)
    SK = ((max_cnt + 127) // 128) * 128
    SK = max(SK, 512)
    NB = (SK + 511) // 512

    scale = np.float32(1.0 / np.sqrt(np.float32(DK)))
    wq8 = (Wq * scale).astype(NP_BF16)
    wk8 = Wk.astype(NP_BF16)
    wv_aug = np.concatenate([Wv, np.zeros((D, 1), np.float32)], axis=1).astype(NP_BF16)
    blob = np.zeros((128, 772), NP_BF16)
    blob[:, 0:256] = wk8.reshape(4, 128, DK).transpose(1, 0, 2).reshape(128, 256)
    blob[:, 256:512] = wq8.reshape(4, 128, DK).transpose(1, 0, 2).reshape(128, 256)
    blob[:, 512:772] = (
        wv_aug.reshape(4, 128, DK + 1).transpose(1, 0, 2).reshape(128, 260)
    )
    wvl_row = np.concatenate([bv, np.ones(1, np.float32)]).astype(NP_BF16)
    blobf = np.zeros((128, 2), np.float32)
    blobf[:, 0] = np.concatenate([bq * scale, bq * scale])
    blobf[:, 1] = np.concatenate([bk, bk])

    in_maps = []
    for b in range(B):
        ix = idxs[b]
        cnt = len(ix)
        ktb = np.zeros((D, SK), NP_BF16)
        vtb = np.zeros((D, SK), NP_BF16)
        ktb[:, :cnt] = k[b][ix].astype(NP_BF16).T
        vtb[:, :cnt] = v[b][ix].astype(NP_BF16).T
        blob2 = np.zeros((1, DK + 1 + SK), NP_BF16)
        blob2[0, 0 : DK + 1] = wvl_row
        blob2[0, DK + 1 : DK + 1 + cnt] = 1.0
        in_maps.append(
            dict(
                qt=_tile_dmajor(np.ascontiguousarray(q[b].astype(NP_BF16).T), QB),
                kt=_tile_dmajor(ktb, NB),
                vt=_tile_dmajor(vtb, NB),
                blob=blob,
                blob2=blob2,
                blobf=blobf,
            )
        )

    if SK not in _NC_CACHE:
        _NC_CACHE[SK] = _build_nc(SK)
    return _NC_CACHE[SK], in_maps


def kernel(**inputs) -> np.ndarray:
    nc, in_maps = prepare(inputs)
    res = run_bass_kernel_spmd(nc, in_maps, list(range(N_CORES)))
    out = np.stack([res.results[b]["out"] for b in range(len(in_maps))], axis=0)
    return out.astype(np.float32)
